# revision 1
# baseline (speedup 1.0000x reference)
"""Trainium2 Bass kernel for nn_BILINEAR_56169582297414 (gnn message passing).

Reference computation (per prediction pair b):
    item_e = item_table[item_inputs[b]]                    # [D]
    mem_e  = user_table[member_ids[b, :]]                  # [M, D]
    scores[m] = mem_e[m] @ W_bil @ item_e + b_bil          # bilinear
    w = scores * member_mask[b]                            # mask padded members
    fu = sum_m w[m] * mem_e[m]                             # [D]
    ne = [fu * item_e, fu, item_e]                         # [3D]
    y = sigmoid(relu(ne @ W1.T + b1) @ W2.T + b2)          # [1]

Strategy: data-parallel over 8 NeuronCores, tables replicated. Embedding
gathers dominate; SWDGE descriptor emission is the serial resource
(~8.5ns/row via batched dma_gather vs ~11.1ns effective via per-128-row
indirect DMA). So:
  - rows are sorted by true group length (desc) and striped across cores so
    all cores share one per-tile max-length profile; each tile fetches only
    maxL members (~1.9x fewer rows);
  - gathers use the GPSIMD dma_gather custom op (mlp library), one
    instruction per 4-tile group, against 4-packed table views
    (user_table.reshape(25000,128); idx=id>>2) with the 1-of-4 sub-row
    select done on DVE via copy_predicated masks (host-precomputed);
  - bilinear projection + MLP head run on TensorE; scores/weighted-sum on
    DVE with broadcast APs. The program is built per input length profile.
"""

import sys

sys.path.insert(0, "/opt/trn_rl_repo")

import numpy as np

B = 262144
M = 16
NU = 100000
NI = 50000
D = 32
N_CORES = 8
BC = B // N_CORES
P = 128
NT = BC // P
G = 4

_COMPILED = {}


def _group_gl(prof, g=G):
    """Per-group max member count (prof is non-increasing)."""
    return [max(prof[i * g : (i + 1) * g]) for i in range(len(prof) // g)]


def build_kernel(bc, nu4, ni2, g=G, prof=None):
    """Build the per-core Bass program against 4-packed user table
    [nu4, 128] and 2-packed item table [ni2, 64]."""
    import concourse.bacc as bacc
    import concourse.tile as tile
    from concourse import mybir
    from concourse.library_config import mlp

    nt = bc // P
    assert nt % g == 0
    ngroups = nt // g
    if prof is None:
        prof = [M] * nt
    prof = [int(max(1, min(M, x))) for x in prof]
    gls = _group_gl(prof, g)
    dt = mybir.dt

    # flat col offsets for per-group idx/mask tensors
    idx_cols = [g * gl * 8 for gl in gls]       # int16 cols ([128, .])
    jm_cols = [g * gl for gl in gls]            # mask cols
    idx_off = np.concatenate([[0], np.cumsum(idx_cols)]).astype(int)
    jm_off = np.concatenate([[0], np.cumsum(jm_cols)]).astype(int)

    nc = bacc.Bacc("TRN2", target_bir_lowering=False, debug=False)

    ids16 = nc.dram_tensor("ids16", [P, int(idx_off[-1])], dt.int16,
                           kind="ExternalInput")
    iid16 = nc.dram_tensor("iid16", [P, ngroups * g * 8], dt.int16,
                           kind="ExternalInput")
    msel = [
        nc.dram_tensor(f"msel{q}", [P, int(jm_off[-1])], dt.uint8,
                       kind="ExternalInput")
        for q in (1, 2, 3)
    ]
    isel1 = nc.dram_tensor("isel1", [P, ngroups * g], dt.uint8,
                           kind="ExternalInput")
    mask = nc.dram_tensor("mask", [bc, M], dt.float32, kind="ExternalInput")
    user4 = nc.dram_tensor("user4", [nu4, 4 * D], dt.float32, kind="ExternalInput")
    item2 = nc.dram_tensor("item2", [ni2, 2 * D], dt.float32, kind="ExternalInput")
    w_bil_t = nc.dram_tensor("w_bil_t", [D, D], dt.float32, kind="ExternalInput")
    w1_t = nc.dram_tensor("w1_t", [3 * D, 8], dt.float32, kind="ExternalInput")
    w2_t = nc.dram_tensor("w2_t", [8, 1], dt.float32, kind="ExternalInput")
    b1 = nc.dram_tensor("b1", [8, 1], dt.float32, kind="ExternalInput")
    b2 = nc.dram_tensor("b2", [1, 1], dt.float32, kind="ExternalInput")
    bbil = nc.dram_tensor("bbil", [P, 1], dt.float32, kind="ExternalInput")
    ident = nc.dram_tensor("ident", [P, P], dt.float32, kind="ExternalInput")
    y_out = nc.dram_tensor("y", [nt, P], dt.float32, kind="ExternalOutput")

    GM = g * M
    GNE = g * 3 * D
    GP = g * P

    with tile.TileContext(nc) as tc:
        with (
            tc.tile_pool(name="const", bufs=1) as cpool,
            tc.tile_pool(name="io", bufs=4) as iopool,
            tc.tile_pool(name="work", bufs=3) as wpool,
            tc.tile_pool(name="gath", bufs=2) as gpool,
            tc.tile_pool(name="prodp", bufs=2) as prpool,
            tc.tile_pool(name="psum", bufs=1, space="PSUM") as ppool,
            tc.tile_pool(name="psumv", bufs=2, space="PSUM") as ppoolv,
        ):
            with tc.tile_critical():
                nc.gpsimd.load_library(mlp)

            wt_sb = cpool.tile([D, D], dt.float32, tag="wt")
            nc.sync.dma_start(out=wt_sb[:], in_=w_bil_t[:])
            w1_sb = cpool.tile([3 * D, 8], dt.float32, tag="w1")
            nc.sync.dma_start(out=w1_sb[:], in_=w1_t[:])
            w2_sb = cpool.tile([8, 1], dt.float32, tag="w2")
            nc.sync.dma_start(out=w2_sb[:], in_=w2_t[:])
            b1_sb = cpool.tile([8, 1], dt.float32, tag="b1")
            nc.sync.dma_start(out=b1_sb[:], in_=b1[:])
            b2_sb = cpool.tile([1, 1], dt.float32, tag="b2")
            nc.sync.dma_start(out=b2_sb[:], in_=b2[:])
            bbil_sb = cpool.tile([P, 1], dt.float32, tag="bbil")
            nc.sync.dma_start(out=bbil_sb[:], in_=bbil[:])
            id_sb = cpool.tile([P, P], dt.float32, tag="ident")
            nc.sync.dma_start(out=id_sb[:], in_=ident[:])

            for gi in range(ngroups):
                r0 = gi * g * P
                GL = gls[gi]
                nmi = g * GL * 128  # member idxs this group

                ids_sb = iopool.tile([P, idx_cols[gi]], dt.int16, tag="ids")
                nc.sync.dma_start(
                    out=ids_sb[:],
                    in_=ids16[:, int(idx_off[gi]) : int(idx_off[gi + 1])],
                )
                iid_sb = iopool.tile([P, g * 8], dt.int16, tag="iid")
                nc.sync.dma_start(
                    out=iid_sb[:],
                    in_=iid16[:, gi * g * 8 : (gi + 1) * g * 8],
                )
                ms0_sb = iopool.tile([P, g * GL], dt.uint8, tag="ms0")
                ms1_sb = iopool.tile([P, g * GL], dt.uint8, tag="ms1")
                ms2_sb = iopool.tile([P, g * GL], dt.uint8, tag="ms2")
                ms_sb = [ms0_sb, ms1_sb, ms2_sb]
                for q in range(3):
                    nc.sync.dma_start(
                        out=ms_sb[q][:],
                        in_=msel[q][:, int(jm_off[gi]) : int(jm_off[gi + 1])],
                    )
                is_sb = iopool.tile([P, g], dt.uint8, tag="is1")
                nc.sync.dma_start(
                    out=is_sb[:], in_=isel1[:, gi * g : (gi + 1) * g]
                )
                mask_sb = iopool.tile([P, GM], dt.float32, tag="mask")
                nc.sync.dma_start(
                    out=mask_sb[:].rearrange("p (g m) -> p g m", g=g),
                    in_=mask[r0 : r0 + g * P, :].rearrange("(g p) m -> p g m", p=P),
                )

                # Batched member gather: idx i=(j*GL+m)*128+p -> block j*GL+m,
                # partition p; each 512B element = 4 packed table rows.
                g4_sb = gpool.tile([P, g * GL * 4 * D], dt.float32, tag="g4")
                g4 = g4_sb[:].rearrange("p (c e) -> p c e", c=g * GL)
                nc.gpsimd.dma_gather(
                    out_ap=g4,
                    in_ap=user4[:],
                    idxs_ap=ids_sb[:],
                    num_idxs=nmi,
                    num_idxs_reg=nmi,
                    elem_size=4 * D,
                    single_packet=False,
                )

                # 1-of-4 sub-row select -> mem [P, (g,GL), D]
                mem_sb = wpool.tile([P, g * GL * D], dt.float32, tag="mem")
                mem3 = mem_sb[:].rearrange("p (c d) -> p c d", c=g * GL)
                nc.vector.tensor_copy(out=mem3, in_=g4[:, :, 0:D])
                for q in range(3):
                    nc.vector.copy_predicated(
                        out=mem3,
                        mask=ms_sb[q][:].unsqueeze(2).broadcast_to([P, g * GL, D]),
                        data=g4[:, :, (q + 1) * D : (q + 2) * D],
                    )

                # Batched item gather (2-packed): idx i=j*128+p
                g2_sb = gpool.tile([P, g * 2 * D], dt.float32, tag="g2")
                g2 = g2_sb[:].rearrange("p (c e) -> p c e", c=g)
                nc.gpsimd.dma_gather(
                    out_ap=g2,
                    in_ap=item2[:],
                    idxs_ap=iid_sb[:],
                    num_idxs=g * 128,
                    num_idxs_reg=g * 128,
                    elem_size=2 * D,
                    single_packet=False,
                )

                ne_sb = wpool.tile([P, GNE], dt.float32, tag="ne")
                ne3 = ne_sb[:].rearrange("p (g c) -> p g c", g=g)
                nc.vector.tensor_copy(
                    out=ne3[:, :, 2 * D : 3 * D], in_=g2[:, :, 0:D]
                )
                nc.vector.copy_predicated(
                    out=ne3[:, :, 2 * D : 3 * D],
                    mask=is_sb[:].unsqueeze(2).broadcast_to([P, g, D]),
                    data=g2[:, :, D : 2 * D],
                )

                itemT_ps = ppool.tile([D, GP], dt.float32, tag="itemT", space="PSUM")
                for j in range(g):
                    nc.tensor.transpose(
                        out=itemT_ps[:, j * P : (j + 1) * P],
                        in_=ne3[:, j, 2 * D : 3 * D],
                        identity=id_sb[:],
                    )
                itemT_sb = wpool.tile([D, GP], dt.float32, tag="itemT")
                nc.scalar.activation(
                    out=itemT_sb[:],
                    in_=itemT_ps[:],
                    func=mybir.ActivationFunctionType.Copy,
                )

                v_ps = ppoolv.tile([P, g * D], dt.float32, tag="v", space="PSUM")
                for j in range(g):
                    nc.tensor.matmul(
                        v_ps[:, j * D : (j + 1) * D],
                        lhsT=itemT_sb[:, j * P : (j + 1) * P],
                        rhs=wt_sb[:],
                        start=True,
                        stop=True,
                    )

                mem4 = mem_sb[:].rearrange("p (g m d) -> p g m d", g=g, m=GL)
                v_b = (
                    v_ps[:]
                    .rearrange("p (g d) -> p g d", g=g)
                    .unsqueeze(2)
                    .broadcast_to([P, g, GL, D])
                )
                prod_sb = prpool.tile([P, GM * D], dt.float32, tag="prod")
                prod4 = prod_sb[:].rearrange("p (g m d) -> p g m d", g=g, m=M)[
                    :, :, :GL, :
                ]
                nc.vector.tensor_mul(out=prod4, in0=mem4, in1=v_b)

                scores_sb = wpool.tile([P, GM], dt.float32, tag="scores")
                sc3 = scores_sb[:].rearrange("p (g m) -> p g m", g=g)
                nc.vector.reduce_sum(
                    out=sc3[:, :, :GL], in_=prod4, axis=mybir.AxisListType.X
                )

                w_sb = wpool.tile([P, GM], dt.float32, tag="w")
                w3 = w_sb[:].rearrange("p (g m) -> p g m", g=g)
                m3 = mask_sb[:].rearrange("p (g m) -> p g m", g=g)
                nc.vector.scalar_tensor_tensor(
                    out=w3[:, :, :GL],
                    in0=sc3[:, :, :GL],
                    scalar=bbil_sb[:, :1],
                    in1=m3[:, :, :GL],
                    op0=mybir.AluOpType.add,
                    op1=mybir.AluOpType.mult,
                )

                w_b = w3[:, :, :GL].unsqueeze(3).broadcast_to([P, g, GL, D])
                nc.vector.tensor_mul(out=prod4, in0=mem4, in1=w_b)

                nc.vector.reduce_sum(
                    out=ne3[:, :, D : 2 * D],
                    in_=prod_sb[:]
                    .rearrange("p (g m d) -> p g d m", g=g, m=M)[:, :, :, :GL],
                    axis=mybir.AxisListType.X,
                )

                nc.vector.tensor_mul(
                    out=ne3[:, :, 0:D],
                    in0=ne3[:, :, D : 2 * D],
                    in1=ne3[:, :, 2 * D : 3 * D],
                )

                neT_ps = ppool.tile([3 * D, GP], dt.float32, tag="neT", space="PSUM")
                for j in range(g):
                    nc.tensor.transpose(
                        out=neT_ps[:, j * P : (j + 1) * P],
                        in_=ne3[:, j, :],
                        identity=id_sb[:],
                    )
                neT_sb = wpool.tile([3 * D, GP], dt.float32, tag="neTs")
                nc.scalar.activation(
                    out=neT_sb[:],
                    in_=neT_ps[:],
                    func=mybir.ActivationFunctionType.Copy,
                )

                hT_ps = ppool.tile([8, GP], dt.float32, tag="hT", space="PSUM")
                for j in range(g):
                    nc.tensor.matmul(
                        hT_ps[:, j * P : (j + 1) * P],
                        lhsT=w1_sb[:],
                        rhs=neT_sb[:, j * P : (j + 1) * P],
                        start=True,
                        stop=True,
                    )
                hT_sb = wpool.tile([8, GP], dt.float32, tag="hTs")
                nc.scalar.activation(
                    out=hT_sb[:],
                    in_=hT_ps[:],
                    func=mybir.ActivationFunctionType.Relu,
                    bias=b1_sb[:, :1],
                )

                yT_ps = ppool.tile([1, GP], dt.float32, tag="yT", space="PSUM")
                for j in range(g):
                    nc.tensor.matmul(
                        yT_ps[:, j * P : (j + 1) * P],
                        lhsT=w2_sb[:],
                        rhs=hT_sb[:, j * P : (j + 1) * P],
                        start=True,
                        stop=True,
                    )
                y_sb = iopool.tile([1, GP], dt.float32, tag="y")
                nc.scalar.activation(
                    out=y_sb[:],
                    in_=yT_ps[:],
                    func=mybir.ActivationFunctionType.Sigmoid,
                    bias=b2_sb[:1, :1],
                )
                nc.sync.dma_start(
                    out=y_out[gi * g : (gi + 1) * g, :], in_=y_sb[:]
                )

    nc.compile()
    return nc


def _lengths_from_mask(mask_b):
    mm = np.asarray(mask_b, dtype=bool)
    pos = np.arange(1, M + 1, dtype=np.int32)
    return (mm * pos[None, :]).max(axis=1).astype(np.int32)


def prepare(item_inputs, member_ids, member_mask, n_cores=N_CORES):
    L = _lengths_from_mask(member_mask)
    order = np.argsort(-L, kind="stable")
    n = len(L)
    bc = n // n_cores
    nt = bc // P
    Ls = L[order]
    prof = [int(max(1, Ls[t * P * n_cores])) for t in range(nt)]
    return order, prof


def _wrap16(idv):
    """[n] int16 idx list -> [128, n/16] wrapped + replicated layout."""
    n = len(idv)
    w16 = idv.reshape(n // 16, 16).T
    return np.tile(w16, (8, 1))


def _make_in_maps(item_inputs, member_ids, member_mask, user_table, item_table,
                  W_bil, b_bil, W1, b1, W2, b2, order, prof, g=G):
    item_inputs = np.asarray(item_inputs).astype(np.int32).reshape(-1)
    member_ids = np.asarray(member_ids).astype(np.int32)
    mask_f = np.asarray(member_mask).astype(np.float32)
    user4 = np.ascontiguousarray(
        np.asarray(user_table, dtype=np.float32).reshape(NU // 4, 4 * D)
    )
    item2 = np.ascontiguousarray(
        np.asarray(item_table, dtype=np.float32).reshape(NI // 2, 2 * D)
    )
    w_bil_t = np.ascontiguousarray(np.asarray(W_bil, dtype=np.float32).T)
    w1_t = np.ascontiguousarray(np.asarray(W1, dtype=np.float32).T)
    w2_t = np.ascontiguousarray(np.asarray(W2, dtype=np.float32).T)
    b1_c = np.asarray(b1, dtype=np.float32).reshape(8, 1)
    b2_c = np.asarray(b2, dtype=np.float32).reshape(1, 1)
    bbil_c = np.full((P, 1), np.asarray(b_bil, dtype=np.float32).reshape(-1)[0],
                     dtype=np.float32)
    ident = np.eye(P, dtype=np.float32)

    gls = _group_gl(prof, g)
    ngroups = len(gls)

    in_maps = []
    for c in range(N_CORES):
        rows = order[c::N_CORES]
        mi = member_ids[rows]              # [bc, M]
        ii = item_inputs[rows]             # [bc]
        idx_parts, m1, m2p, m3p, ip, is1 = [], [], [], [], [], []
        for gi in range(ngroups):
            GL = gls[gi]
            blk = mi[gi * g * P : (gi + 1) * g * P, :GL]     # [g*P, GL]
            b4 = blk.reshape(g, P, GL)
            idv = np.transpose(b4, (0, 2, 1)).reshape(-1)     # (j,m,p) order
            idx_parts.append(_wrap16((idv >> 2).astype(np.int16)))
            sub = (np.transpose(b4, (0, 2, 1)) & 3)           # [g, GL, P]
            subm = np.transpose(sub, (2, 0, 1)).reshape(P, g * GL)  # [p,(j,m)]
            m1.append((subm == 1).astype(np.uint8))
            m2p.append((subm == 2).astype(np.uint8))
            m3p.append((subm == 3).astype(np.uint8))
            ib = ii[gi * g * P : (gi + 1) * g * P].reshape(g, P)
            iv = ib.reshape(-1)                                # (j,p) order
            ip.append(_wrap16((iv >> 1).astype(np.int16)))
            is1.append(((ib & 1).T).astype(np.uint8))        # [P, g]
        in_maps.append({
            "ids16": np.concatenate(idx_parts, axis=1),
            "iid16": np.concatenate(ip, axis=1),
            "msel1": np.concatenate(m1, axis=1),
            "msel2": np.concatenate(m2p, axis=1),
            "msel3": np.concatenate(m3p, axis=1),
            "isel1": np.concatenate(is1, axis=1),
            "mask": np.ascontiguousarray(mask_f[rows]),
            "user4": user4,
            "item2": item2,
            "w_bil_t": w_bil_t,
            "w1_t": w1_t,
            "w2_t": w2_t,
            "b1": b1_c,
            "b2": b2_c,
            "bbil": bbil_c,
            "ident": ident,
        })
    return in_maps


def _get_compiled(prof):
    key = tuple(prof)
    if key not in _COMPILED:
        _COMPILED[key] = build_kernel(BC, NU // 4, NI // 2, G, prof=list(prof))
    return _COMPILED[key]


def run_on_hw(nc, in_maps, trace=False):
    from concourse import bass_utils

    return bass_utils.run_bass_kernel_spmd(
        nc, in_maps, core_ids=list(range(N_CORES)), trace=trace
    )


def kernel(item_inputs, member_ids, member_mask, user_table, item_table,
           W_bil, b_bil, W1, b1, W2, b2):
    order, prof = prepare(item_inputs, member_ids, member_mask)
    nc = _get_compiled(prof)
    in_maps = _make_in_maps(item_inputs, member_ids, member_mask, user_table,
                            item_table, W_bil, b_bil, W1, b1, W2, b2, order, prof)
    res = run_on_hw(nc, in_maps, trace=False)
    y = np.empty(B, dtype=np.float32)
    for c in range(N_CORES):
        y[order[c::N_CORES]] = res.results[c]["y"].reshape(BC)
    return y.reshape(B, 1)



# revision 2
# speedup vs baseline: 2.4332x; 2.4332x over previous
"""Trainium2 Bass kernel for nn_BILINEAR_56169582297414 (gnn message passing).

Reference computation (per prediction pair b):
    item_e = item_table[item_inputs[b]]                    # [D]
    mem_e  = user_table[member_ids[b, :]]                  # [M, D]
    scores[m] = mem_e[m] @ W_bil @ item_e + b_bil          # bilinear
    w = scores * member_mask[b]                            # mask padded members
    fu = sum_m w[m] * mem_e[m]                             # [D]
    ne = [fu * item_e, fu, item_e]                         # [3D]
    y = sigmoid(relu(ne @ W1.T + b1) @ W2.T + b2)          # [1]

Strategy: data-parallel over 8 NeuronCores, tables replicated. The serial
resource is SWDGE gather descriptor processing (~10 ns/desc on one queue,
~3.5-4 ns/desc aggregate on 4 queues). So:
  - rows sorted by true group length (desc) and striped across cores; each
    tile fetches only its max length (~1.9x fewer member rows);
  - member gathers (bf16 4-packed user table, 256B elems) and item gathers
    (fp32 2-packed item table, 512B elems) are spread round-robin over all
    4 SWDGE queues (num_swdge_queues=4) so all 4 Q7 core pairs emit
    descriptors concurrently;
  - the 1-of-4 / 1-of-2 packed-row selects use copy_predicated only (the
    plain DVE COPY opcode is ~4x slower than COPY_PREDICATED for the same
    shape); member math runs in bf16 (2x DVE);
  - the weighted member sum uses a pairwise add tree over contiguous views
    (the strided [g,d,m] TENSOR_REDUCE it replaces ran ~20x slower);
  - bilinear projection + MLP head run on TensorE in fp32 as before.
"""

import sys

sys.path.insert(0, "/opt/trn_rl_repo")

import numpy as np

B = 262144
M = 16
NU = 100000
NI = 50000
D = 32
N_CORES = 8
BC = B // N_CORES
P = 128
NT = BC // P
G = 4

_COMPILED = {}


def _group_gl(prof, g=G):
    """Per-group max member count (prof is non-increasing)."""
    return [max(prof[i * g : (i + 1) * g]) for i in range(len(prof) // g)]


def _next_pow2(x):
    h = 1
    while h < x:
        h *= 2
    return h


def build_kernel(bc, g=G, prof=None):
    """Build the per-core Bass program against 4-packed bf16 user table
    [25000, 128] and 2-packed fp32 item table [25000, 64]."""
    import concourse.bacc as bacc
    import concourse.tile as tile
    from concourse import mybir
    from concourse.library_config import mlp

    nt = bc // P
    assert nt % g == 0
    ngroups = nt // g
    if prof is None:
        prof = [M] * nt
    prof = [int(max(1, min(M, x))) for x in prof]
    gls = _group_gl(prof, g)
    dt = mybir.dt

    # flat col offsets for per-group idx/mask tensors
    idx_cols = [g * gl * 8 for gl in gls]       # int16 cols ([128, .])
    jm_cols = [g * gl for gl in gls]            # mask cols
    idx_off = np.concatenate([[0], np.cumsum(idx_cols)]).astype(int)
    jm_off = np.concatenate([[0], np.cumsum(jm_cols)]).astype(int)

    nc = bacc.Bacc("TRN2", target_bir_lowering=False, debug=False,
                   num_swdge_queues=4)

    ids16 = nc.dram_tensor("ids16", [P, int(idx_off[-1])], dt.int16,
                           kind="ExternalInput")
    iid16 = nc.dram_tensor("iid16", [P, ngroups * g * 8], dt.int16,
                           kind="ExternalInput")
    msel = [
        nc.dram_tensor(f"msel{q}", [P, int(jm_off[-1])], dt.uint8,
                       kind="ExternalInput")
        for q in (0, 1, 2, 3)
    ]
    isel = [
        nc.dram_tensor(f"isel{q}", [P, ngroups * g], dt.uint8,
                       kind="ExternalInput")
        for q in (0, 1)
    ]
    mask = nc.dram_tensor("mask", [bc, M], dt.float32, kind="ExternalInput")
    user4 = nc.dram_tensor("user4", [NU // 4, 4 * D], dt.bfloat16,
                           kind="ExternalInput")
    item2 = nc.dram_tensor("item2", [NI // 2, 2 * D], dt.float32,
                           kind="ExternalInput")
    w_bil_t = nc.dram_tensor("w_bil_t", [D, D], dt.float32, kind="ExternalInput")
    w1_t = nc.dram_tensor("w1_t", [3 * D, 8], dt.float32, kind="ExternalInput")
    w2_t = nc.dram_tensor("w2_t", [8, 1], dt.float32, kind="ExternalInput")
    b1 = nc.dram_tensor("b1", [8, 1], dt.float32, kind="ExternalInput")
    b2 = nc.dram_tensor("b2", [1, 1], dt.float32, kind="ExternalInput")
    bbil = nc.dram_tensor("bbil", [P, 1], dt.float32, kind="ExternalInput")
    ident = nc.dram_tensor("ident", [P, P], dt.float32, kind="ExternalInput")
    y_out = nc.dram_tensor("y", [nt, P], dt.float32, kind="ExternalOutput")

    GM = g * M
    GNE = g * 3 * D
    GP = g * P

    with tile.TileContext(nc) as tc:
        with (
            tc.tile_pool(name="const", bufs=1) as cpool,
            tc.tile_pool(name="io", bufs=6) as iopool,
            tc.tile_pool(name="work", bufs=3) as wpool,
            tc.tile_pool(name="gath", bufs=5) as gpool,
            tc.tile_pool(name="prodp", bufs=2) as prpool,
            tc.tile_pool(name="psum", bufs=1, space="PSUM") as ppool,
            tc.tile_pool(name="psumv", bufs=2, space="PSUM") as ppoolv,
        ):
            with tc.tile_critical():
                nc.gpsimd.load_library(mlp)

            wt_sb = cpool.tile([D, D], dt.float32, tag="wt")
            nc.sync.dma_start(out=wt_sb[:], in_=w_bil_t[:])
            w1_sb = cpool.tile([3 * D, 8], dt.float32, tag="w1")
            nc.sync.dma_start(out=w1_sb[:], in_=w1_t[:])
            w2_sb = cpool.tile([8, 1], dt.float32, tag="w2")
            nc.sync.dma_start(out=w2_sb[:], in_=w2_t[:])
            b1_sb = cpool.tile([8, 1], dt.float32, tag="b1")
            nc.sync.dma_start(out=b1_sb[:], in_=b1[:])
            b2_sb = cpool.tile([1, 1], dt.float32, tag="b2")
            nc.sync.dma_start(out=b2_sb[:], in_=b2[:])
            bbil_sb = cpool.tile([P, 1], dt.float32, tag="bbil")
            nc.sync.dma_start(out=bbil_sb[:], in_=bbil[:])
            id_sb = cpool.tile([P, P], dt.float32, tag="ident")
            nc.sync.dma_start(out=id_sb[:], in_=ident[:])

            for gi in range(ngroups):
                r0 = gi * g * P
                GL = gls[gi]
                Hp = _next_pow2(GL)
                nmi = g * GL * 128  # member idxs this group

                ids_sb = iopool.tile([P, idx_cols[gi]], dt.int16, tag="ids")
                nc.sync.dma_start(
                    out=ids_sb[:],
                    in_=ids16[:, int(idx_off[gi]) : int(idx_off[gi + 1])],
                )
                iid_sb = iopool.tile([P, g * 8], dt.int16, tag="iid")
                nc.sync.dma_start(
                    out=iid_sb[:],
                    in_=iid16[:, gi * g * 8 : (gi + 1) * g * 8],
                )
                ms_sb = []
                for q in range(4):
                    t = iopool.tile([P, g * GL], dt.uint8, tag=f"ms{q}")
                    nc.sync.dma_start(
                        out=t[:],
                        in_=msel[q][:, int(jm_off[gi]) : int(jm_off[gi + 1])],
                    )
                    ms_sb.append(t)
                is_sb = []
                for q in range(2):
                    t = iopool.tile([P, g], dt.uint8, tag=f"is{q}")
                    nc.sync.dma_start(
                        out=t[:], in_=isel[q][:, gi * g : (gi + 1) * g]
                    )
                    is_sb.append(t)
                mask_sb = iopool.tile([P, GM], dt.float32, tag="mask")
                nc.sync.dma_start(
                    out=mask_sb[:].rearrange("p (g m) -> p g m", g=g),
                    in_=mask[r0 : r0 + g * P, :].rearrange("(g p) m -> p g m", p=P),
                )

                # Batched member gather: idx i=(j*GL+m)*128+p -> block j*GL+m,
                # partition p; each 256B element = 4 packed bf16 table rows.
                g4_sb = gpool.tile([P, g * GL * 4 * D], dt.bfloat16, tag="g4")
                g4 = g4_sb[:].rearrange("p (c e) -> p c e", c=g * GL)
                nc.gpsimd.dma_gather(
                    out_ap=g4,
                    in_ap=user4[:],
                    idxs_ap=ids_sb[:],
                    num_idxs=nmi,
                    num_idxs_reg=nmi,
                    elem_size=4 * D,
                    single_packet=False,
                    queue_num=gi % 4,
                )

                # 1-of-4 sub-row select -> mem [P, (g,GL), D] bf16
                mem_sb = wpool.tile([P, g * GL * D], dt.bfloat16, tag="mem")
                mem3 = mem_sb[:].rearrange("p (c d) -> p c d", c=g * GL)
                for q in range(4):
                    nc.vector.copy_predicated(
                        out=mem3,
                        mask=ms_sb[q][:].unsqueeze(2).broadcast_to([P, g * GL, D]),
                        data=g4[:, :, q * D : (q + 1) * D],
                    )

                # Batched item gather (2-packed fp32): idx i=j*128+p
                g2_sb = gpool.tile([P, g * 2 * D], dt.float32, tag="g2")
                g2 = g2_sb[:].rearrange("p (c e) -> p c e", c=g)
                nc.gpsimd.dma_gather(
                    out_ap=g2,
                    in_ap=item2[:],
                    idxs_ap=iid_sb[:],
                    num_idxs=g * 128,
                    num_idxs_reg=g * 128,
                    elem_size=2 * D,
                    single_packet=False,
                    queue_num=(gi + 2) % 4,
                )

                ne_sb = wpool.tile([P, GNE], dt.float32, tag="ne")
                ne3 = ne_sb[:].rearrange("p (g c) -> p g c", g=g)
                for q in range(2):
                    nc.vector.copy_predicated(
                        out=ne3[:, :, 2 * D : 3 * D],
                        mask=is_sb[q][:].unsqueeze(2).broadcast_to([P, g, D]),
                        data=g2[:, :, q * D : (q + 1) * D],
                    )

                itemT_ps = ppool.tile([D, GP], dt.float32, tag="itemT", space="PSUM")
                for j in range(g):
                    nc.tensor.transpose(
                        out=itemT_ps[:, j * P : (j + 1) * P],
                        in_=ne3[:, j, 2 * D : 3 * D],
                        identity=id_sb[:],
                    )
                itemT_sb = wpool.tile([D, GP], dt.float32, tag="itemT")
                nc.scalar.activation(
                    out=itemT_sb[:],
                    in_=itemT_ps[:],
                    func=mybir.ActivationFunctionType.Copy,
                )

                v_ps = ppoolv.tile([P, g * D], dt.float32, tag="v", space="PSUM")
                for j in range(g):
                    nc.tensor.matmul(
                        v_ps[:, j * D : (j + 1) * D],
                        lhsT=itemT_sb[:, j * P : (j + 1) * P],
                        rhs=wt_sb[:],
                        start=True,
                        stop=True,
                    )
                v16_sb = wpool.tile([P, g * D], dt.bfloat16, tag="v16")
                nc.scalar.activation(
                    out=v16_sb[:],
                    in_=v_ps[:],
                    func=mybir.ActivationFunctionType.Copy,
                )

                mem4 = mem_sb[:].rearrange("p (g m d) -> p g m d", g=g, m=GL)
                v_b = (
                    v16_sb[:]
                    .rearrange("p (g d) -> p g d", g=g)
                    .unsqueeze(2)
                    .broadcast_to([P, g, GL, D])
                )
                prod_sb = prpool.tile([P, GM * D], dt.bfloat16, tag="prod")
                prod4 = prod_sb[:].rearrange("p (g m d) -> p g m d", g=g, m=M)[
                    :, :, :GL, :
                ]
                nc.vector.tensor_mul(out=prod4, in0=mem4, in1=v_b)

                scores_sb = wpool.tile([P, GM], dt.float32, tag="scores")
                sc3 = scores_sb[:].rearrange("p (g m) -> p g m", g=g)
                nc.vector.reduce_sum(
                    out=sc3[:, :, :GL], in_=prod4, axis=mybir.AxisListType.X
                )

                w16_sb = wpool.tile([P, GM], dt.bfloat16, tag="w16")
                w3 = w16_sb[:].rearrange("p (g m) -> p g m", g=g)
                m3 = mask_sb[:].rearrange("p (g m) -> p g m", g=g)
                nc.vector.scalar_tensor_tensor(
                    out=w3[:, :, :GL],
                    in0=sc3[:, :, :GL],
                    scalar=bbil_sb[:, :1],
                    in1=m3[:, :, :GL],
                    op0=mybir.AluOpType.add,
                    op1=mybir.AluOpType.mult,
                )

                # weighted members; zero-pad to next pow2 for the add tree
                wp_sb = prpool.tile([P, GM * D], dt.bfloat16, tag="wprod")
                wp4 = wp_sb[:].rearrange("p (g m d) -> p g m d", g=g, m=M)
                w_b = w3[:, :, :GL].unsqueeze(3).broadcast_to([P, g, GL, D])
                nc.vector.tensor_mul(out=wp4[:, :, :GL, :], in0=mem4, in1=w_b)
                if Hp > GL:
                    nc.vector.memset(wp4[:, :, GL:Hp, :], 0)

                fu_out = ne3[:, :, D : 2 * D]
                if Hp == 1:
                    nc.vector.tensor_copy(out=fu_out, in_=wp4[:, :, 0, :])
                elif Hp == 2:
                    nc.vector.tensor_add(
                        out=fu_out, in0=wp4[:, :, 0, :], in1=wp4[:, :, 1, :]
                    )
                else:
                    t_sb = prpool.tile([P, g * 12 * D], dt.float32, tag="tree")
                    tv = t_sb[:].rearrange("p (g m d) -> p g m d", g=g, m=12)
                    if Hp == 4:
                        nc.vector.tensor_add(
                            out=tv[:, :, 0:2, :],
                            in0=wp4[:, :, 0:2, :],
                            in1=wp4[:, :, 2:4, :],
                        )
                        nc.vector.tensor_add(
                            out=fu_out, in0=tv[:, :, 0, :], in1=tv[:, :, 1, :]
                        )
                    elif Hp == 8:
                        nc.vector.tensor_add(
                            out=tv[:, :, 0:4, :],
                            in0=wp4[:, :, 0:4, :],
                            in1=wp4[:, :, 4:8, :],
                        )
                        nc.vector.tensor_add(
                            out=tv[:, :, 4:6, :],
                            in0=tv[:, :, 0:2, :],
                            in1=tv[:, :, 2:4, :],
                        )
                        nc.vector.tensor_add(
                            out=fu_out, in0=tv[:, :, 4, :], in1=tv[:, :, 5, :]
                        )
                    else:  # Hp == 16
                        nc.vector.tensor_add(
                            out=tv[:, :, 0:8, :],
                            in0=wp4[:, :, 0:8, :],
                            in1=wp4[:, :, 8:16, :],
                        )
                        nc.vector.tensor_add(
                            out=tv[:, :, 8:12, :],
                            in0=tv[:, :, 0:4, :],
                            in1=tv[:, :, 4:8, :],
                        )
                        nc.vector.tensor_add(
                            out=tv[:, :, 0:2, :],
                            in0=tv[:, :, 8:10, :],
                            in1=tv[:, :, 10:12, :],
                        )
                        nc.vector.tensor_add(
                            out=fu_out, in0=tv[:, :, 0, :], in1=tv[:, :, 1, :]
                        )

                nc.vector.tensor_mul(
                    out=ne3[:, :, 0:D],
                    in0=ne3[:, :, D : 2 * D],
                    in1=ne3[:, :, 2 * D : 3 * D],
                )

                neT_ps = ppool.tile([3 * D, GP], dt.float32, tag="neT", space="PSUM")
                for j in range(g):
                    nc.tensor.transpose(
                        out=neT_ps[:, j * P : (j + 1) * P],
                        in_=ne3[:, j, :],
                        identity=id_sb[:],
                    )
                neT_sb = wpool.tile([3 * D, GP], dt.float32, tag="neTs")
                nc.scalar.activation(
                    out=neT_sb[:],
                    in_=neT_ps[:],
                    func=mybir.ActivationFunctionType.Copy,
                )

                hT_ps = ppool.tile([8, GP], dt.float32, tag="hT", space="PSUM")
                for j in range(g):
                    nc.tensor.matmul(
                        hT_ps[:, j * P : (j + 1) * P],
                        lhsT=w1_sb[:],
                        rhs=neT_sb[:, j * P : (j + 1) * P],
                        start=True,
                        stop=True,
                    )
                hT_sb = wpool.tile([8, GP], dt.float32, tag="hTs")
                nc.scalar.activation(
                    out=hT_sb[:],
                    in_=hT_ps[:],
                    func=mybir.ActivationFunctionType.Relu,
                    bias=b1_sb[:, :1],
                )

                yT_ps = ppool.tile([1, GP], dt.float32, tag="yT", space="PSUM")
                for j in range(g):
                    nc.tensor.matmul(
                        yT_ps[:, j * P : (j + 1) * P],
                        lhsT=w2_sb[:],
                        rhs=hT_sb[:, j * P : (j + 1) * P],
                        start=True,
                        stop=True,
                    )
                y_sb = iopool.tile([1, GP], dt.float32, tag="y")
                nc.scalar.activation(
                    out=y_sb[:],
                    in_=yT_ps[:],
                    func=mybir.ActivationFunctionType.Sigmoid,
                    bias=b2_sb[:1, :1],
                )
                nc.sync.dma_start(
                    out=y_out[gi * g : (gi + 1) * g, :], in_=y_sb[:]
                )

    nc.compile()
    return nc


def _lengths_from_mask(mask_b):
    mm = np.asarray(mask_b, dtype=bool)
    pos = np.arange(1, M + 1, dtype=np.int32)
    return (mm * pos[None, :]).max(axis=1).astype(np.int32)


def prepare(item_inputs, member_ids, member_mask, n_cores=N_CORES):
    L = _lengths_from_mask(member_mask)
    order = np.argsort(-L, kind="stable")
    n = len(L)
    bc = n // n_cores
    nt = bc // P
    Ls = L[order]
    prof = [int(max(1, Ls[t * P * n_cores])) for t in range(nt)]
    return order, prof


def _wrap16(idv):
    """[n] int16 idx list -> [128, n/16] wrapped + replicated layout."""
    n = len(idv)
    w16 = idv.reshape(n // 16, 16).T
    return np.tile(w16, (8, 1))


def _make_in_maps(item_inputs, member_ids, member_mask, user_table, item_table,
                  W_bil, b_bil, W1, b1, W2, b2, order, prof, g=G):
    import ml_dtypes

    bf16 = ml_dtypes.bfloat16
    item_inputs = np.asarray(item_inputs).astype(np.int32).reshape(-1)
    member_ids = np.asarray(member_ids).astype(np.int32)
    mask_f = np.asarray(member_mask).astype(np.float32)
    user4 = np.ascontiguousarray(
        np.asarray(user_table, dtype=np.float32).astype(bf16).reshape(
            NU // 4, 4 * D
        )
    )
    item2 = np.ascontiguousarray(
        np.asarray(item_table, dtype=np.float32).reshape(NI // 2, 2 * D)
    )
    w_bil_t = np.ascontiguousarray(np.asarray(W_bil, dtype=np.float32).T)
    w1_t = np.ascontiguousarray(np.asarray(W1, dtype=np.float32).T)
    w2_t = np.ascontiguousarray(np.asarray(W2, dtype=np.float32).T)
    b1_c = np.asarray(b1, dtype=np.float32).reshape(8, 1)
    b2_c = np.asarray(b2, dtype=np.float32).reshape(1, 1)
    bbil_c = np.full((P, 1), np.asarray(b_bil, dtype=np.float32).reshape(-1)[0],
                     dtype=np.float32)
    ident = np.eye(P, dtype=np.float32)

    gls = _group_gl(prof, g)
    ngroups = len(gls)

    in_maps = []
    for c in range(N_CORES):
        rows = order[c::N_CORES]
        mi = member_ids[rows]              # [bc, M]
        ii = item_inputs[rows]             # [bc]
        idx_parts, ip = [], []
        msel_p = [[], [], [], []]
        isel_p = [[], []]
        for gi in range(ngroups):
            GL = gls[gi]
            blk = mi[gi * g * P : (gi + 1) * g * P, :GL]     # [g*P, GL]
            b4 = blk.reshape(g, P, GL)
            idv = np.transpose(b4, (0, 2, 1)).reshape(-1)     # (j,m,p) order
            idx_parts.append(_wrap16((idv >> 2).astype(np.int16)))
            sub = (np.transpose(b4, (0, 2, 1)) & 3)           # [g, GL, P]
            subm = np.transpose(sub, (2, 0, 1)).reshape(P, g * GL)  # [p,(j,m)]
            for q in range(4):
                msel_p[q].append((subm == q).astype(np.uint8))
            ib = ii[gi * g * P : (gi + 1) * g * P].reshape(g, P)
            iv = ib.reshape(-1)                                # (j,p) order
            ip.append(_wrap16((iv >> 1).astype(np.int16)))
            ibit = ((ib & 1).T).astype(np.uint8)               # [P, g]
            isel_p[0].append(1 - ibit)
            isel_p[1].append(ibit)
        im = {
            "ids16": np.concatenate(idx_parts, axis=1),
            "iid16": np.concatenate(ip, axis=1),
            "mask": np.ascontiguousarray(mask_f[rows]),
            "user4": user4,
            "item2": item2,
            "w_bil_t": w_bil_t,
            "w1_t": w1_t,
            "w2_t": w2_t,
            "b1": b1_c,
            "b2": b2_c,
            "bbil": bbil_c,
            "ident": ident,
        }
        for q in range(4):
            im[f"msel{q}"] = np.concatenate(msel_p[q], axis=1)
        for q in range(2):
            im[f"isel{q}"] = np.concatenate(isel_p[q], axis=1)
        in_maps.append(im)
    return in_maps


def _get_compiled(prof):
    key = tuple(prof)
    if key not in _COMPILED:
        _COMPILED[key] = build_kernel(BC, G, prof=list(prof))
    return _COMPILED[key]


def run_on_hw(nc, in_maps, trace=False):
    from concourse import bass_utils

    return bass_utils.run_bass_kernel_spmd(
        nc, in_maps, core_ids=list(range(N_CORES)), trace=trace
    )


def kernel(item_inputs, member_ids, member_mask, user_table, item_table,
           W_bil, b_bil, W1, b1, W2, b2):
    order, prof = prepare(item_inputs, member_ids, member_mask)
    nc = _get_compiled(prof)
    in_maps = _make_in_maps(item_inputs, member_ids, member_mask, user_table,
                            item_table, W_bil, b_bil, W1, b1, W2, b2, order, prof)
    res = run_on_hw(nc, in_maps, trace=False)
    y = np.empty(B, dtype=np.float32)
    for c in range(N_CORES):
        y[order[c::N_CORES]] = res.results[c]["y"].reshape(BC)
    return y.reshape(B, 1)


# revision 3
# speedup vs baseline: 2.7352x; 1.1241x over previous
"""Trainium2 Bass kernel for nn_BILINEAR_56169582297414 (gnn message passing).

Reference computation (per prediction pair b):
    item_e = item_table[item_inputs[b]]                    # [D]
    mem_e  = user_table[member_ids[b, :]]                  # [M, D]
    scores[m] = mem_e[m] @ W_bil @ item_e + b_bil          # bilinear
    w = scores * member_mask[b]                            # mask padded members
    fu = sum_m w[m] * mem_e[m]                             # [D]
    ne = [fu * item_e, fu, item_e]                         # [3D]
    y = sigmoid(relu(ne @ W1.T + b1) @ W2.T + b2)          # [1]

Strategy: data-parallel over 8 NeuronCores, tables replicated. The serial
resource is SWDGE gather descriptor processing (~10 ns/desc per queue).
  - rows sorted by true group length (desc) and striped across cores; each
    tile fetches only its max length (~1.9x fewer member rows);
  - member gathers (bf16 4-packed user table, 256B elems) spread over the
    4 SWDGE queues (4 Q7 core pairs); item gathers (fp32 2-packed, 512B)
    batched 4 groups per instruction to amortize per-instruction overhead;
  - dynamic_dma_scratch_size doubled so each queue's descriptor ring holds
    ~2 gathers, reducing decode-stage await_space head-of-line blocking;
  - packed-row selects via copy_predicated only (DVE COPY is ~4x slower),
    on uint32-bitcast views to halve element count; member math in bf16;
  - the weighted member sum uses a pairwise add tree over contiguous views
    (the strided [g,d,m] TENSOR_REDUCE it replaces ran ~20x slower);
  - bilinear projection + MLP head on TensorE in fp32; the per-group MLP
    matmuls run once over all 4 tiles (512-wide rhs).
"""

import sys

sys.path.insert(0, "/opt/trn_rl_repo")

import numpy as np

B = 262144
M = 16
NU = 100000
NI = 50000
D = 32
N_CORES = 8
BC = B // N_CORES
P = 128
NT = BC // P
G = 4

_COMPILED = {}


def _group_gl(prof, g=G):
    """Per-group max member count (prof is non-increasing)."""
    return [max(prof[i * g : (i + 1) * g]) for i in range(len(prof) // g)]


def _next_pow2(x):
    h = 1
    while h < x:
        h *= 2
    return h


def build_kernel(bc, g=G, prof=None):
    import concourse.bacc as bacc
    import concourse.tile as tile
    from concourse import mybir
    from concourse.library_config import mlp

    nt = bc // P
    assert nt % g == 0
    ngroups = nt // g
    assert ngroups % 4 == 0
    if prof is None:
        prof = [M] * nt
    prof = [int(max(1, min(M, x))) for x in prof]
    gls = _group_gl(prof, g)
    dt = mybir.dt

    # flat col offsets for per-group idx / mask-blob tensors
    idx_cols = [g * gl * 8 for gl in gls]            # int16 cols ([128, .])
    blob_cols = [4 * g * gl + 2 * g for gl in gls]   # uint8 cols: 4 msel + 2 isel
    idx_off = np.concatenate([[0], np.cumsum(idx_cols)]).astype(int)
    blob_off = np.concatenate([[0], np.cumsum(blob_cols)]).astype(int)

    nc = bacc.Bacc("TRN2", target_bir_lowering=False, debug=False,
                   num_swdge_queues=4, dynamic_dma_scratch_size=32768)

    ids16 = nc.dram_tensor("ids16", [P, int(idx_off[-1])], dt.int16,
                           kind="ExternalInput")
    iid16 = nc.dram_tensor("iid16", [P, (ngroups // 4) * 16 * 8], dt.int16,
                           kind="ExternalInput")
    mblob = nc.dram_tensor("mblob", [P, int(blob_off[-1])], dt.uint8,
                           kind="ExternalInput")
    mask = nc.dram_tensor("mask", [bc, M], dt.float32, kind="ExternalInput")
    user4 = nc.dram_tensor("user4", [NU // 4, 4 * D], dt.bfloat16,
                           kind="ExternalInput")
    item2 = nc.dram_tensor("item2", [NI // 2, 2 * D], dt.float32,
                           kind="ExternalInput")
    w_bil_t = nc.dram_tensor("w_bil_t", [D, D], dt.float32, kind="ExternalInput")
    w1_t = nc.dram_tensor("w1_t", [3 * D, 8], dt.float32, kind="ExternalInput")
    w2_t = nc.dram_tensor("w2_t", [8, 1], dt.float32, kind="ExternalInput")
    b1 = nc.dram_tensor("b1", [8, 1], dt.float32, kind="ExternalInput")
    b2 = nc.dram_tensor("b2", [1, 1], dt.float32, kind="ExternalInput")
    bbil = nc.dram_tensor("bbil", [P, 1], dt.float32, kind="ExternalInput")
    ident = nc.dram_tensor("ident", [P, P], dt.float32, kind="ExternalInput")
    y_out = nc.dram_tensor("y", [nt, P], dt.float32, kind="ExternalOutput")

    GM = g * M
    GNE = g * 3 * D
    GP = g * P

    with tile.TileContext(nc) as tc:
        with (
            tc.tile_pool(name="const", bufs=1) as cpool,
            tc.tile_pool(name="io", bufs=6) as iopool,
            tc.tile_pool(name="work", bufs=3) as wpool,
            tc.tile_pool(name="gath", bufs=5) as gpool,
            tc.tile_pool(name="quad", bufs=2) as qpool,
            tc.tile_pool(name="prodp", bufs=2) as prpool,
            tc.tile_pool(name="psum", bufs=1, space="PSUM") as ppool,
            tc.tile_pool(name="psumv", bufs=2, space="PSUM") as ppoolv,
        ):
            with tc.tile_critical():
                nc.gpsimd.load_library(mlp)

            wt_sb = cpool.tile([D, D], dt.float32, tag="wt")
            nc.sync.dma_start(out=wt_sb[:], in_=w_bil_t[:])
            w1_sb = cpool.tile([3 * D, 8], dt.float32, tag="w1")
            nc.sync.dma_start(out=w1_sb[:], in_=w1_t[:])
            w2_sb = cpool.tile([8, 1], dt.float32, tag="w2")
            nc.sync.dma_start(out=w2_sb[:], in_=w2_t[:])
            b1_sb = cpool.tile([8, 1], dt.float32, tag="b1")
            nc.sync.dma_start(out=b1_sb[:], in_=b1[:])
            b2_sb = cpool.tile([1, 1], dt.float32, tag="b2")
            nc.sync.dma_start(out=b2_sb[:], in_=b2[:])
            bbil_sb = cpool.tile([P, 1], dt.float32, tag="bbil")
            nc.sync.dma_start(out=bbil_sb[:], in_=bbil[:])
            id_sb = cpool.tile([P, P], dt.float32, tag="ident")
            nc.sync.dma_start(out=id_sb[:], in_=ident[:])

            quad_sb = None
            for gi in range(ngroups):
                r0 = gi * g * P
                GL = gls[gi]
                Hp = _next_pow2(GL)
                nmi = g * GL * 128  # member idxs this group

                if gi % 4 == 0:
                    qk = gi // 4
                    iid_sb = iopool.tile([P, 16 * 8], dt.int16, tag="iid")
                    nc.sync.dma_start(
                        out=iid_sb[:],
                        in_=iid16[:, qk * 128 : (qk + 1) * 128],
                    )
                    quad_sb = qpool.tile([P, 16 * 2 * D], dt.float32, tag="q2")
                    q2 = quad_sb[:].rearrange("p (c e) -> p c e", c=16)
                    nc.gpsimd.dma_gather(
                        out_ap=q2,
                        in_ap=item2[:],
                        idxs_ap=iid_sb[:],
                        num_idxs=16 * 128,
                        num_idxs_reg=16 * 128,
                        elem_size=2 * D,
                        single_packet=False,
                        queue_num=(qk + 1) % 4,
                    )

                ids_sb = iopool.tile([P, idx_cols[gi]], dt.int16, tag="ids")
                nc.sync.dma_start(
                    out=ids_sb[:],
                    in_=ids16[:, int(idx_off[gi]) : int(idx_off[gi + 1])],
                )

                # Batched member gather: idx i=(j*GL+m)*128+p -> block j*GL+m,
                # partition p; each 256B element = 4 packed bf16 table rows.
                g4_sb = gpool.tile([P, g * GL * 4 * D], dt.bfloat16, tag="g4")
                g4 = g4_sb[:].rearrange("p (c e) -> p c e", c=g * GL)
                nc.gpsimd.dma_gather(
                    out_ap=g4,
                    in_ap=user4[:],
                    idxs_ap=ids_sb[:],
                    num_idxs=nmi,
                    num_idxs_reg=nmi,
                    elem_size=4 * D,
                    single_packet=False,
                    queue_num=gi % 4,
                )

                blob_sb = iopool.tile([P, blob_cols[gi]], dt.uint8, tag="blob")
                nc.sync.dma_start(
                    out=blob_sb[:],
                    in_=mblob[:, int(blob_off[gi]) : int(blob_off[gi + 1])],
                )
                ms_sb = [
                    blob_sb[:, q * g * GL : (q + 1) * g * GL] for q in range(4)
                ]
                is_sb = [
                    blob_sb[:, 4 * g * GL + q * g : 4 * g * GL + (q + 1) * g]
                    for q in range(2)
                ]
                mask_sb = iopool.tile([P, GM], dt.float32, tag="mask")
                nc.sync.dma_start(
                    out=mask_sb[:].rearrange("p (g m) -> p g m", g=g),
                    in_=mask[r0 : r0 + g * P, :].rearrange("(g p) m -> p g m", p=P),
                )

                # 1-of-4 sub-row select on uint32 views -> mem [P,(g,GL),D] bf16
                mem_sb = wpool.tile([P, g * GL * D], dt.bfloat16, tag="mem")
                g4u = (
                    g4_sb[:]
                    .bitcast(dt.uint32)
                    .rearrange("p (c e) -> p c e", c=g * GL)
                )
                memu = (
                    mem_sb[:]
                    .bitcast(dt.uint32)
                    .rearrange("p (c e) -> p c e", c=g * GL)
                )
                HD = D // 2  # uint32 words per row
                for q in range(4):
                    nc.vector.copy_predicated(
                        out=memu,
                        mask=ms_sb[q].unsqueeze(2).broadcast_to([P, g * GL, HD]),
                        data=g4u[:, :, q * HD : (q + 1) * HD],
                    )

                ne_sb = wpool.tile([P, GNE], dt.float32, tag="ne")
                ne3 = ne_sb[:].rearrange("p (g c) -> p g c", g=g)
                q2v = quad_sb[:].rearrange("p (c e) -> p c e", c=16)
                qbase = (gi % 4) * g
                for q in range(2):
                    nc.vector.copy_predicated(
                        out=ne3[:, :, 2 * D : 3 * D],
                        mask=is_sb[q].unsqueeze(2).broadcast_to([P, g, D]),
                        data=q2v[:, qbase : qbase + g, q * D : (q + 1) * D],
                    )

                itemT_ps = ppool.tile([D, GP], dt.float32, tag="itemT", space="PSUM")
                for j in range(g):
                    nc.tensor.transpose(
                        out=itemT_ps[:, j * P : (j + 1) * P],
                        in_=ne3[:, j, 2 * D : 3 * D],
                        identity=id_sb[:],
                    )
                itemT_sb = wpool.tile([D, GP], dt.float32, tag="itemT")
                nc.scalar.activation(
                    out=itemT_sb[:],
                    in_=itemT_ps[:],
                    func=mybir.ActivationFunctionType.Copy,
                )

                v_ps = ppoolv.tile([P, g * D], dt.float32, tag="v", space="PSUM")
                for j in range(g):
                    nc.tensor.matmul(
                        v_ps[:, j * D : (j + 1) * D],
                        lhsT=itemT_sb[:, j * P : (j + 1) * P],
                        rhs=wt_sb[:],
                        start=True,
                        stop=True,
                    )
                v16_sb = wpool.tile([P, g * D], dt.bfloat16, tag="v16")
                nc.scalar.activation(
                    out=v16_sb[:],
                    in_=v_ps[:],
                    func=mybir.ActivationFunctionType.Copy,
                )

                mem4 = mem_sb[:].rearrange("p (g m d) -> p g m d", g=g, m=GL)
                v_b = (
                    v16_sb[:]
                    .rearrange("p (g d) -> p g d", g=g)
                    .unsqueeze(2)
                    .broadcast_to([P, g, GL, D])
                )
                prod_sb = prpool.tile([P, GM * D], dt.bfloat16, tag="prod")
                prod4 = prod_sb[:].rearrange("p (g m d) -> p g m d", g=g, m=M)[
                    :, :, :GL, :
                ]
                nc.vector.tensor_mul(out=prod4, in0=mem4, in1=v_b)

                scores_sb = wpool.tile([P, GM], dt.float32, tag="scores")
                sc3 = scores_sb[:].rearrange("p (g m) -> p g m", g=g)
                nc.vector.reduce_sum(
                    out=sc3[:, :, :GL], in_=prod4, axis=mybir.AxisListType.X
                )

                w16_sb = wpool.tile([P, GM], dt.bfloat16, tag="w16")
                w3 = w16_sb[:].rearrange("p (g m) -> p g m", g=g)
                m3 = mask_sb[:].rearrange("p (g m) -> p g m", g=g)
                nc.vector.scalar_tensor_tensor(
                    out=w3[:, :, :GL],
                    in0=sc3[:, :, :GL],
                    scalar=bbil_sb[:, :1],
                    in1=m3[:, :, :GL],
                    op0=mybir.AluOpType.add,
                    op1=mybir.AluOpType.mult,
                )

                # weighted members; zero-pad to next pow2 for the add tree
                wp_sb = prpool.tile([P, GM * D], dt.bfloat16, tag="wprod")
                wp4 = wp_sb[:].rearrange("p (g m d) -> p g m d", g=g, m=M)
                w_b = w3[:, :, :GL].unsqueeze(3).broadcast_to([P, g, GL, D])
                nc.vector.tensor_mul(out=wp4[:, :, :GL, :], in0=mem4, in1=w_b)
                if Hp > GL:
                    nc.vector.memset(wp4[:, :, GL:Hp, :], 0)

                fu_out = ne3[:, :, D : 2 * D]
                if Hp == 1:
                    nc.vector.tensor_copy(out=fu_out, in_=wp4[:, :, 0, :])
                elif Hp == 2:
                    nc.vector.tensor_add(
                        out=fu_out, in0=wp4[:, :, 0, :], in1=wp4[:, :, 1, :]
                    )
                else:
                    t_sb = prpool.tile([P, g * 12 * D], dt.float32, tag="tree")
                    tv = t_sb[:].rearrange("p (g m d) -> p g m d", g=g, m=12)
                    if Hp == 4:
                        nc.vector.tensor_add(
                            out=tv[:, :, 0:2, :],
                            in0=wp4[:, :, 0:2, :],
                            in1=wp4[:, :, 2:4, :],
                        )
                        nc.vector.tensor_add(
                            out=fu_out, in0=tv[:, :, 0, :], in1=tv[:, :, 1, :]
                        )
                    elif Hp == 8:
                        nc.vector.tensor_add(
                            out=tv[:, :, 0:4, :],
                            in0=wp4[:, :, 0:4, :],
                            in1=wp4[:, :, 4:8, :],
                        )
                        nc.vector.tensor_add(
                            out=tv[:, :, 4:6, :],
                            in0=tv[:, :, 0:2, :],
                            in1=tv[:, :, 2:4, :],
                        )
                        nc.vector.tensor_add(
                            out=fu_out, in0=tv[:, :, 4, :], in1=tv[:, :, 5, :]
                        )
                    else:  # Hp == 16
                        nc.vector.tensor_add(
                            out=tv[:, :, 0:8, :],
                            in0=wp4[:, :, 0:8, :],
                            in1=wp4[:, :, 8:16, :],
                        )
                        nc.vector.tensor_add(
                            out=tv[:, :, 8:12, :],
                            in0=tv[:, :, 0:4, :],
                            in1=tv[:, :, 4:8, :],
                        )
                        nc.vector.tensor_add(
                            out=tv[:, :, 0:2, :],
                            in0=tv[:, :, 8:10, :],
                            in1=tv[:, :, 10:12, :],
                        )
                        nc.vector.tensor_add(
                            out=fu_out, in0=tv[:, :, 0, :], in1=tv[:, :, 1, :]
                        )

                nc.vector.tensor_mul(
                    out=ne3[:, :, 0:D],
                    in0=ne3[:, :, D : 2 * D],
                    in1=ne3[:, :, 2 * D : 3 * D],
                )

                neT_ps = ppool.tile([3 * D, GP], dt.float32, tag="neT", space="PSUM")
                for j in range(g):
                    nc.tensor.transpose(
                        out=neT_ps[:, j * P : (j + 1) * P],
                        in_=ne3[:, j, :],
                        identity=id_sb[:],
                    )
                neT_sb = wpool.tile([3 * D, GP], dt.float32, tag="neTs")
                nc.scalar.activation(
                    out=neT_sb[:],
                    in_=neT_ps[:],
                    func=mybir.ActivationFunctionType.Copy,
                )

                hT_ps = ppool.tile([8, GP], dt.float32, tag="hT", space="PSUM")
                nc.tensor.matmul(
                    hT_ps[:],
                    lhsT=w1_sb[:],
                    rhs=neT_sb[:],
                    start=True,
                    stop=True,
                )
                hT_sb = wpool.tile([8, GP], dt.float32, tag="hTs")
                nc.scalar.activation(
                    out=hT_sb[:],
                    in_=hT_ps[:],
                    func=mybir.ActivationFunctionType.Relu,
                    bias=b1_sb[:, :1],
                )

                yT_ps = ppool.tile([1, GP], dt.float32, tag="yT", space="PSUM")
                nc.tensor.matmul(
                    yT_ps[:],
                    lhsT=w2_sb[:],
                    rhs=hT_sb[:],
                    start=True,
                    stop=True,
                )
                y_sb = iopool.tile([1, GP], dt.float32, tag="y")
                nc.scalar.activation(
                    out=y_sb[:],
                    in_=yT_ps[:],
                    func=mybir.ActivationFunctionType.Sigmoid,
                    bias=b2_sb[:1, :1],
                )
                nc.sync.dma_start(
                    out=y_out[gi * g : (gi + 1) * g, :], in_=y_sb[:]
                )

    nc.compile()
    return nc


def _lengths_from_mask(mask_b):
    mm = np.asarray(mask_b, dtype=bool)
    pos = np.arange(1, M + 1, dtype=np.int32)
    return (mm * pos[None, :]).max(axis=1).astype(np.int32)


def prepare(item_inputs, member_ids, member_mask, n_cores=N_CORES):
    L = _lengths_from_mask(member_mask)
    order = np.argsort(-L, kind="stable")
    n = len(L)
    bc = n // n_cores
    nt = bc // P
    Ls = L[order]
    prof = [int(max(1, Ls[t * P * n_cores])) for t in range(nt)]
    return order, prof


def _wrap16(idv):
    """[n] int16 idx list -> [128, n/16] wrapped + replicated layout."""
    n = len(idv)
    w16 = idv.reshape(n // 16, 16).T
    return np.tile(w16, (8, 1))


def _make_in_maps(item_inputs, member_ids, member_mask, user_table, item_table,
                  W_bil, b_bil, W1, b1, W2, b2, order, prof, g=G):
    import ml_dtypes

    bf16 = ml_dtypes.bfloat16
    item_inputs = np.asarray(item_inputs).astype(np.int32).reshape(-1)
    member_ids = np.asarray(member_ids).astype(np.int32)
    mask_f = np.asarray(member_mask).astype(np.float32)
    user4 = np.ascontiguousarray(
        np.asarray(user_table, dtype=np.float32).astype(bf16).reshape(
            NU // 4, 4 * D
        )
    )
    item2 = np.ascontiguousarray(
        np.asarray(item_table, dtype=np.float32).reshape(NI // 2, 2 * D)
    )
    w_bil_t = np.ascontiguousarray(np.asarray(W_bil, dtype=np.float32).T)
    w1_t = np.ascontiguousarray(np.asarray(W1, dtype=np.float32).T)
    w2_t = np.ascontiguousarray(np.asarray(W2, dtype=np.float32).T)
    b1_c = np.asarray(b1, dtype=np.float32).reshape(8, 1)
    b2_c = np.asarray(b2, dtype=np.float32).reshape(1, 1)
    bbil_c = np.full((P, 1), np.asarray(b_bil, dtype=np.float32).reshape(-1)[0],
                     dtype=np.float32)
    ident = np.eye(P, dtype=np.float32)

    gls = _group_gl(prof, g)
    ngroups = len(gls)

    in_maps = []
    for c in range(N_CORES):
        rows = order[c::N_CORES]
        mi = member_ids[rows]              # [bc, M]
        ii = item_inputs[rows]             # [bc]
        idx_parts, blob_parts, iid_parts = [], [], []
        for gi in range(ngroups):
            GL = gls[gi]
            blk = mi[gi * g * P : (gi + 1) * g * P, :GL]     # [g*P, GL]
            b4 = blk.reshape(g, P, GL)
            idv = np.transpose(b4, (0, 2, 1)).reshape(-1)     # (j,m,p) order
            idx_parts.append(_wrap16((idv >> 2).astype(np.int16)))
            sub = (np.transpose(b4, (0, 2, 1)) & 3)           # [g, GL, P]
            subm = np.transpose(sub, (2, 0, 1)).reshape(P, g * GL)  # [p,(j,m)]
            ib = ii[gi * g * P : (gi + 1) * g * P].reshape(g, P)
            ibit = ((ib & 1).T).astype(np.uint8)               # [P, g]
            blob_parts.append(np.concatenate(
                [(subm == q).astype(np.uint8) for q in range(4)]
                + [1 - ibit, ibit], axis=1))
            iid_parts.append(((ib >> 1).astype(np.int16)))     # [g, P]
        # item idxs per quad of 4 groups: (grp_in_quad, j, p) order
        iid_quads = []
        for qk in range(ngroups // 4):
            iv = np.concatenate(
                [iid_parts[4 * qk + t].reshape(-1) for t in range(4)]
            )
            iid_quads.append(_wrap16(iv))
        im = {
            "ids16": np.concatenate(idx_parts, axis=1),
            "iid16": np.concatenate(iid_quads, axis=1),
            "mblob": np.concatenate(blob_parts, axis=1),
            "mask": np.ascontiguousarray(mask_f[rows]),
            "user4": user4,
            "item2": item2,
            "w_bil_t": w_bil_t,
            "w1_t": w1_t,
            "w2_t": w2_t,
            "b1": b1_c,
            "b2": b2_c,
            "bbil": bbil_c,
            "ident": ident,
        }
        in_maps.append(im)
    return in_maps


def _get_compiled(prof):
    key = tuple(prof)
    if key not in _COMPILED:
        _COMPILED[key] = build_kernel(BC, G, prof=list(prof))
    return _COMPILED[key]


def run_on_hw(nc, in_maps, trace=False):
    from concourse import bass_utils

    return bass_utils.run_bass_kernel_spmd(
        nc, in_maps, core_ids=list(range(N_CORES)), trace=trace
    )


def kernel(item_inputs, member_ids, member_mask, user_table, item_table,
           W_bil, b_bil, W1, b1, W2, b2):
    order, prof = prepare(item_inputs, member_ids, member_mask)
    nc = _get_compiled(prof)
    in_maps = _make_in_maps(item_inputs, member_ids, member_mask, user_table,
                            item_table, W_bil, b_bil, W1, b1, W2, b2, order, prof)
    res = run_on_hw(nc, in_maps, trace=False)
    y = np.empty(B, dtype=np.float32)
    for c in range(N_CORES):
        y[order[c::N_CORES]] = res.results[c]["y"].reshape(BC)
    return y.reshape(B, 1)
